# revision 10
# baseline (speedup 1.0000x reference)
"""BiasAdd + LayerNorm + FP8 quantization kernel for Trainium2 (Bass/Tile).

Reference computation (see problem reference.py):
    bda  = residual + (x + bias)                 # [B,S,H] -> flattened [B*S, H]
    ln   = layernorm(bda) * ln_weight + ln_bias  # fp32
    amax = max(|ln|)
    fp8  = clip(ln, +-448).astype(float8_e4m3fn)
    returns (bda2, fp8, amax)

Sharding: data-parallel over the flattened token dim (32768 rows) across
8 NeuronCores -> 4096 rows/core. bias/ln_weight/ln_bias replicated.
x and residual shards are stacked host-side into one [2, R, H] input so
each supertile needs a single load DMA (keeps per-instruction semaphore
wait counts within the ISA limit). amax: per-partition partial maxima
[128] per core, final max on host.

Per-core HBM traffic: 32 MiB read + 20 MiB write = 52 MiB, which at the
~358 GB/s per-NC HBM limit gives a ~152 us roofline (memory regime).
"""

import sys

import numpy as np

_TRN_REPO = "/opt/trn_rl_repo"
if _TRN_REPO not in sys.path:
    sys.path.insert(0, _TRN_REPO)

import ml_dtypes  # noqa: E402
import concourse.bass as bass  # noqa: E402
import concourse.bacc as bacc  # noqa: E402
import concourse.tile as tile  # noqa: E402
from concourse import mybir  # noqa: E402

EPS = 1e-5
H = 1024
P = 128
N_CORES = 8
R_FULL = 8 * 4096  # B * S
R_CORE = R_FULL // N_CORES  # 4096 rows per core
BLOCKS_PER_TILE = 2  # A: 128-row blocks per supertile (shared host/kernel)


def build_nc(rows: int = R_CORE, blocks_per_tile: int = BLOCKS_PER_TILE) -> bass.Bass:
    """One NeuronCore's program: bias-add + layernorm + fp8 over [rows, H]."""
    dt = mybir.dt
    A = blocks_per_tile
    assert rows % (P * A) == 0
    nblk = rows // P
    J = nblk // A

    # Bacc (not plain Bass): its finalize() runs generate_event_semaphores(),
    # which splits multi-semaphore waits to satisfy the 1-wait-per-instruction
    # hardware constraint that Tile-emitted code otherwise violates.
    nc = bacc.Bacc()
    # x and residual interleaved per supertile: rows ordered
    # [x-tile-0 (A*P rows), res-tile-0, x-tile-1, res-tile-1, ...]
    xr = nc.declare_dram_parameter("xr", [2 * rows, H], dt.float32, isOutput=False)
    bias = nc.declare_dram_parameter("bias", [H], dt.float32, isOutput=False)
    gamma = nc.declare_dram_parameter("ln_weight", [H], dt.float32, isOutput=False)
    beta = nc.declare_dram_parameter("ln_bias", [H], dt.float32, isOutput=False)
    bda_out = nc.declare_dram_parameter("bda_out", [rows, H], dt.float32, isOutput=True)
    fp8_out = nc.declare_dram_parameter("fp8_out", [rows, H], dt.float8e4, isOutput=True)
    amax_out = nc.declare_dram_parameter("amax_out", [P, 1], dt.float32, isOutput=True)

    # viewed as J supertiles of [P partitions, 2 planes, A blocks, H]
    xv = xr[:].rearrange("(j t a p) h -> j p t a h", t=2, p=P, a=A)
    bv = bda_out[:].rearrange("(j a p) h -> j p a h", p=P, a=A)
    fv = fp8_out[:].rearrange("(j a p) h -> j p a h", p=P, a=A)

    with tile.TileContext(nc) as tc:
        with (
            tc.tile_pool(name="consts", bufs=1) as consts,
            tc.tile_pool(name="io", bufs=3) as io,
            tc.tile_pool(name="work", bufs=3) as work,
            tc.tile_pool(name="stats", bufs=4) as stats,
        ):
            # Broadcast the three H-vectors across all 128 partitions once.
            bias_b = consts.tile([P, H], dt.float32, tag="bias_b")
            gamma_b = consts.tile([P, H], dt.float32, tag="gamma_b")
            beta_b = consts.tile([P, H], dt.float32, tag="beta_b")
            for tgt, src in ((bias_b, bias), (gamma_b, gamma), (beta_b, beta)):
                src_ap = src[:]
                bcast = bass.AP(
                    tensor=src_ap.tensor,
                    offset=src_ap.offset,
                    ap=[[0, P]] + list(src_ap.ap),
                )
                nc.sync.dma_start(out=tgt, in_=bcast)
            eps_t = consts.tile([P, 1], dt.float32, tag="eps")
            nc.vector.memset(eps_t, EPS)
            # Per-partition running |ln| maxima, one column per row-block.
            amax_acc = consts.tile([P, nblk], dt.float32, tag="amax_acc")

            for j in range(J):
                xt = io.tile([P, 2, A, H], dt.float32, tag="xt")
                nc.sync.dma_start(out=xt, in_=xv[j])
                lnt = work.tile([P, A, H], dt.float32, tag="ln")
                f8t = work.tile([P, A, H], dt.float8e4, tag="f8")
                for a in range(A):
                    xs = xt[:, 0, a, :]
                    # bda = x + residual + bias (written back into the x plane)
                    nc.vector.tensor_add(out=xs, in0=xs, in1=xt[:, 1, a, :])
                    nc.vector.tensor_add(out=xs, in0=xs, in1=bias_b)
                    # mean/var via bn_stats over two 512-wide subgroups
                    st = stats.tile([P, 2, 6], dt.float32, tag="st")
                    xss = xs.rearrange("p (n f) -> p n f", f=512)
                    nc.vector.bn_stats(out=st[:, 0, :], in_=xss[:, 0, :])
                    nc.vector.bn_stats(out=st[:, 1, :], in_=xss[:, 1, :])
                    mv = stats.tile([P, 2], dt.float32, tag="mv")
                    nc.vector.bn_aggr(out=mv, in_=st)
                    mu = mv[:, 0:1]
                    var = mv[:, 1:2]
                    # rsigma = 1/sqrt(var + eps)
                    sig = stats.tile([P, 1], dt.float32, tag="sig")
                    nc.scalar.activation(
                        out=sig,
                        in_=var,
                        func=mybir.ActivationFunctionType.Sqrt,
                        bias=eps_t,
                        scale=1.0,
                    )
                    nc.vector.reciprocal(out=sig, in_=sig)
                    # t = (bda - mu) * rsigma
                    ls = lnt[:, a, :]
                    nc.vector.tensor_scalar(
                        out=ls,
                        in0=xs,
                        scalar1=mu,
                        scalar2=sig,
                        op0=mybir.AluOpType.subtract,
                        op1=mybir.AluOpType.mult,
                    )
                    # t *= gamma
                    nc.vector.tensor_mul(out=ls, in0=ls, in1=gamma_b)
                    # ln = t + beta, then per-partition |ln| max into its column
                    ablk = j * A + a
                    nc.vector.tensor_add(out=ls, in0=ls, in1=beta_b)
                    nc.vector.tensor_reduce(
                        out=amax_acc[:, ablk : ablk + 1],
                        in_=ls,
                        axis=mybir.AxisListType.X,
                        op=mybir.AluOpType.max,
                        apply_absolute_value=True,
                    )
                    # fp8 e4m3 cast on the scalar engine
                    nc.scalar.copy(out=f8t[:, a, :], in_=ls)
                nc.sync.dma_start(out=bv[j], in_=xt[:, 0, :, :])
                nc.sync.dma_start(out=fv[j], in_=f8t)

            amax_pp = stats.tile([P, 1], dt.float32, tag="apc")
            nc.vector.tensor_reduce(
                out=amax_pp,
                in_=amax_acc,
                axis=mybir.AxisListType.X,
                op=mybir.AluOpType.max,
            )
            nc.sync.dma_start(out=amax_out[:, :], in_=amax_pp)
    # Run the Bacc compile passes (register allocation, event-semaphore
    # legalization); run_bass_via_pjrt serializes nc.m as-is.
    nc.finalize()
    return nc


_NC_CACHE: dict = {}


def _get_nc() -> bass.Bass:
    if "nc" not in _NC_CACHE:
        _NC_CACHE["nc"] = build_nc()
    return _NC_CACHE["nc"]


def _run(in_maps, trace=False, **kwargs):
    from concourse.bass_utils import run_bass_kernel_spmd

    return run_bass_kernel_spmd(
        _get_nc(), in_maps, list(range(N_CORES)), trace=trace, **kwargs
    )


def _make_in_maps(x, bias, residual, ln_weight, ln_bias):
    x2 = np.asarray(x, dtype=np.float32).reshape(R_FULL, H)
    r2 = np.asarray(residual, dtype=np.float32).reshape(R_FULL, H)
    bias = np.ascontiguousarray(np.asarray(bias, dtype=np.float32))
    w = np.ascontiguousarray(np.asarray(ln_weight, dtype=np.float32))
    b = np.ascontiguousarray(np.asarray(ln_bias, dtype=np.float32))
    tile_rows = BLOCKS_PER_TILE * P
    n_tiles = R_CORE // tile_rows
    in_maps = []
    for i in range(N_CORES):
        rows = slice(i * R_CORE, (i + 1) * R_CORE)
        xs_ = x2[rows].reshape(n_tiles, tile_rows, H)
        rs_ = r2[rows].reshape(n_tiles, tile_rows, H)
        xrs = np.stack([xs_, rs_], axis=1).reshape(2 * R_CORE, H)
        in_maps.append(
            {"xr": xrs, "bias": bias, "ln_weight": w, "ln_bias": b}
        )
    return in_maps


def _gather(results):
    bda = np.concatenate([r["bda_out"] for r in results], axis=0)
    fp8 = np.concatenate([r["fp8_out"] for r in results], axis=0)
    # TRN float8e4 matches OCP e4m3fn bit-for-bit over its finite range.
    fp8 = fp8.view(ml_dtypes.float8_e4m3fn)
    amax = np.float32(max(np.max(r["amax_out"]) for r in results))
    return bda, fp8, amax


def kernel(x, bias, residual, ln_weight, ln_bias):
    in_maps = _make_in_maps(x, bias, residual, ln_weight, ln_bias)
    out = _run(in_maps)
    return _gather(out.results)
